# revision 1
# baseline (speedup 1.0000x reference)
"""Trainium2 Bass kernel for nn_AtenMmQuint8: quint8 dense matmul.

    out = ((x - 65) * 0.199) @ ((y - 160) * 0.0215)
    x: [2048, 4096] int32 (quint8 values 0..255)
    y: [4096, 2048] int32 (quint8 values 0..255)
    out: [2048, 2048] fp32

Sharding: 4x2 tensor-parallel grid over the 8 NeuronCores (4 M-blocks x
2 N-blocks). This halves per-core HBM traffic vs. the 1x8 column-only
split, and keeps each core's matmul work identical (256 PE matmuls).

Host staging: the inputs are quint8 tensors boxed in int32; we stage them
to the device in their natural 1-byte storage, and stage x K-major
(transposed) so the PE's stationary operand needs no on-chip transpose
(DMA transpose only supports 2-byte dtypes).

Device kernel (identical SPMD program on all 8 cores):
  - K is interleaved across SBUF partitions (k = p*32 + j) so each
    load-chunk DMA is 128 large contiguous runs (one per partition)
    instead of 128*nk sub-1KB ones; HWDGE descriptor generation is
    ~4ns/descriptor, so the naive layout serializes the whole load
    stream. The contraction is a permutation of K applied identically
    to both operands, so the matmul result is unchanged.
  - Progressively-sized load chunks (x on the SP HWDGE ring, y on the
    ACT ring, in parallel) into persistent u8 SBUF buffers; per-ring DMA
    completions serialize at ~2.2us each, so small leading chunks start
    the pipeline and big trailing chunks amortize.
  - Dequant bias casts on VectorE: bf16 <- u8 + (-zero_point).
    (q - zp) is an integer in [-160, 190] -> exactly representable in
    bf16, so the matmul is exact to fp32 accumulation order.
  - PE prewarm: ~30 throwaway matmuls while the first loads are in
    flight release the HAM clock gate (1.2 -> 2.4 GHz) just as the real
    stream starts.
  - PE matmul bf16 x bf16 -> fp32 at the 215ns/matmul roofline,
    accumulating the whole 512x1024 block across all 8 PSUM banks
    k-outer (PE never waits on a full K pass); the last 8 k-tiles run
    m-major so banks retire early and their copy+store overlaps the
    remaining matmuls.
  - Scale+copy PSUM -> SBUF fused with the combined scale on VectorE,
    one store DMA per 128-row group (the last split in two so the
    kernel-ending chain is short).
"""

import numpy as np

import concourse.bass as bass  # noqa: F401  (kept for callers/debugging)
import concourse.mybir as mybir
import concourse.tile as tile
from concourse import bacc
from concourse.bass_utils import run_bass_kernel_spmd

X_ZP, Y_ZP = 65.0, 160.0
SCALE = 0.199 * 0.0215

M, K, N = 2048, 4096, 2048
GM, GN = 4, 2  # core grid: 4 M-blocks x 2 N-blocks
MC, NC = M // GM, N // GN  # 512 x 1024 per-core output block
P = 128  # partitions / k-tile size
NB = 512  # psum bank free size (one fp32 bank; matmul cannot cross banks)
# k-tiles per load DMA and per dequant-cast op. Per-ring DMA
# completions serialize at ~2.2us, so small leading chunks start the
# pipeline early and big trailing chunks amortize; cast ranges nest
# inside DMA ranges so a cast waits on exactly one transfer.
DMA_CHUNKS = (1, 1, 2, 4, 8, 8, 8)
SW_BULK = 0  # SWDGE bulk path measured slower; disabled
CAST_CHUNKS = (1, 1, 2, 4, 4, 4, 4, 4, 4, 4)
KT_TAIL = 8  # trailing k-tiles run m-major so PSUM banks retire early
N_WARM = 30


def _emit(tc, xT, ys, out, dma_chunks=DMA_CHUNKS, cast_chunks=CAST_CHUNKS,
          kt_tail=KT_TAIL, n_warm=N_WARM, sw_bulk=SW_BULK):
    """Emit the per-core device program.

    xT: [k, mc] u8 DRAM (x slice, K-major), ys: [k, nnc] u8 DRAM,
    out: [mc, nnc] fp32 DRAM.
    """
    nc = tc.nc
    k, mc = xT.shape
    nnc = ys.shape[1]
    kt = k // P
    mt = mc // P
    nt = nnc // NB
    assert sum(dma_chunks) + sw_bulk == kt and sum(cast_chunks) == kt

    fp32 = mybir.dt.float32
    bf16 = mybir.dt.bfloat16
    u8 = mybir.dt.uint8

    with (
        tc.tile_pool(name="sb", bufs=1) as sbp,
        tc.tile_pool(name="osb", bufs=mt, space="SBUF") as osbp,
        tc.tile_pool(name="ps", bufs=mt * nt, space="PSUM") as psp,
    ):
        # Everything is persistent (fits in SBUF at this problem size):
        # each DMA / cast writes a disjoint slice, so instructions don't
        # accrue buffer-recycling waits.
        xu = sbp.tile([P, kt, mc], u8, name="xu")
        yu = sbp.tile([P, kt, nnc], u8, name="yu")
        xba = sbp.tile([P, kt, mc], bf16, name="xba")
        yba = sbp.tile([P, kt, nnc], bf16, name="yba")
        wt = sbp.tile([P, P], bf16, name="wt")
        psum = [
            [psp.tile([P, NB], fp32, tag="ps", name=f"ps_{m}_{n}") for n in range(nt)]
            for m in range(mt)
        ]

        # HAM prewarm: the PE sits idle for ~4 us while the first chunks
        # load+cast; throwaway matmuls release the clock gate to 8/8
        # before the real stream starts.
        nc.gpsimd.memset(wt[:], 0.0)
        for _ in range(n_warm):
            nc.tensor.matmul(psum[0][0][:, :P], wt[:], wt[:], start=True, stop=True)

        # K is interleaved across partitions (k = p*kt + j): each
        # partition's j-range is then one big contiguous DRAM run, so a
        # chunk DMA is 128 descriptors (one per partition) instead of
        # 128*nk 0.5-1KB ones -- HWDGE descriptor generation (~4ns/desc)
        # otherwise serializes the whole load stream. The contraction is
        # a permutation of K, identical for x and y, so the matmul sum
        # is unchanged.
        xTr = xT.rearrange("(p j) m -> p j m", j=kt)
        ysr = ys.rearrange("(p j) n -> p j n", j=kt)
        # Bulk tail of the loads rides SWDGE (gpsimd) -- a third DMA path
        # running in parallel with both HWDGE rings. Its slower software
        # issue/completion doesn't matter for data the PE only needs
        # ~25us later, and it keeps the HWDGE rings' ~2.2us/DMA
        # completion slots for the latency-critical early chunks.
        if sw_bulk:
            sw0 = kt - sw_bulk
            nc.gpsimd.dma_start(yu[:, sw0:kt, :], ysr[:, sw0:kt, :])
            nc.gpsimd.dma_start(xu[:, sw0:kt, :], xTr[:, sw0:kt, :])
        k0 = 0
        for nk in dma_chunks:
            nc.sync.dma_start(xu[:, k0 : k0 + nk, :], xTr[:, k0 : k0 + nk, :])
            # y-loads issue from the ACT HWDGE ring, in parallel with the
            # x-load issues on the SP ring.
            nc.scalar.dma_start(yu[:, k0 : k0 + nk, :], ysr[:, k0 : k0 + nk, :])
            k0 += nk

        k0 = 0
        for nk in cast_chunks:
            sl = slice(k0, k0 + nk)
            nc.vector.tensor_scalar_add(xba[:, sl, :], xu[:, sl, :], -X_ZP)
            nc.vector.tensor_scalar_add(yba[:, sl, :], yu[:, sl, :], -Y_ZP)
            k0 += nk

        def mm(j, m, n):
            nc.tensor.matmul(
                psum[m][n][:],
                xba[:, j, m * P : (m + 1) * P],
                yba[:, j, n * NB : (n + 1) * NB],
                start=(j == 0),
                stop=(j == kt - 1),
            )

        # k-outer: touch every psum bank each k-tile so the PE stream
        # stays dense while loads/casts race ahead.
        for j in range(kt - kt_tail):
            for m in range(mt):
                for n in range(nt):
                    mm(j, m, n)
        # m-outer tail: bank group m finishes its K accumulation early so
        # its copy+store overlaps the remaining matmuls.
        for m in range(mt):
            for j in range(kt - kt_tail, kt):
                for n in range(nt):
                    mm(j, m, n)

        # Scale+copy PSUM->SBUF on VectorE (ACT stays DMA-issue only, no
        # activation-table load), one store per 128-row group (contiguous
        # rows of `out`).
        for m in range(mt):
            osb = osbp.tile([P, nnc], fp32, tag="osb", name=f"osb_{m}")
            for n in range(nt):
                nc.vector.tensor_scalar_mul(
                    osb[:, n * NB : (n + 1) * NB], psum[m][n][:], SCALE
                )
            if m < mt - 1:
                nc.sync.dma_start(out[m * P : (m + 1) * P, :], osb[:])
            else:
                # split the last row-group's store so the kernel-ending
                # chain (last matmul -> copy -> store) is half as long
                for n in range(nt):
                    nc.sync.dma_start(
                        out[m * P : (m + 1) * P, n * NB : (n + 1) * NB],
                        osb[:, n * NB : (n + 1) * NB],
                    )


def _build_nc(k=K, mc=MC, nnc=NC, **emit_kw):
    nc = bacc.Bacc("TRN2", target_bir_lowering=False, debug=False)
    xT = nc.declare_dram_parameter("xT", [k, mc], mybir.dt.uint8, isOutput=False)
    ys = nc.declare_dram_parameter("ys", [k, nnc], mybir.dt.uint8, isOutput=False)
    out = nc.declare_dram_parameter("out", [mc, nnc], mybir.dt.float32, isOutput=True)
    with tile.TileContext(nc) as tc:
        _emit(tc, xT[:], ys[:], out[:], **emit_kw)
    nc.compile()
    return nc


_CACHE = {}


def _get_nc():
    if "nc" not in _CACHE:
        _CACHE["nc"] = _build_nc()
    return _CACHE["nc"]


def kernel(x, y):
    x = np.asarray(x)
    y = np.asarray(y)
    assert x.shape == (M, K) and y.shape == (K, N)
    # quint8 payload boxed in int32 (guaranteed 0..255 by the problem spec);
    # stage in natural 1-byte storage, x in K-major layout.
    xT_u8 = x.T.astype(np.uint8)
    y_u8 = y.astype(np.uint8)

    in_maps = []
    for i in range(GM * GN):
        mi, ni = divmod(i, GN)
        in_maps.append(
            {
                "xT": np.ascontiguousarray(xT_u8[:, mi * MC : (mi + 1) * MC]),
                "ys": np.ascontiguousarray(y_u8[:, ni * NC : (ni + 1) * NC]),
            }
        )

    res = run_bass_kernel_spmd(_get_nc(), in_maps, list(range(GM * GN)))
    _CACHE["last_results"] = res

    out = np.empty((M, N), np.float32)
    for i in range(GM * GN):
        mi, ni = divmod(i, GN)
        out[mi * MC : (mi + 1) * MC, ni * NC : (ni + 1) * NC] = res.results[i]["out"]
    return out



# revision 2
# speedup vs baseline: 1.5516x; 1.5516x over previous
"""Trainium2 Bass kernel for nn_AtenMmQuint8: quint8 dense matmul via fp8.

    out = ((x - 65) * 0.199) @ ((y - 160) * 0.0215)
    x: [2048, 4096] int32 (quint8 values 0..255)
    y: [4096, 2048] int32 (quint8 values 0..255)
    out: [2048, 2048] fp32

Sharding: 4x2 tensor-parallel grid over the 8 NeuronCores (4 M-blocks x
2 N-blocks); per-core output block 512x1024, identical SPMD program.

Math: the PE runs fp8e4m3 matmuls in DoubleRow (double-pumped) mode at
2x the bf16 rate (157 vs 78.6 TF/s), halving the PE roofline from
~55us to ~27.3us per core.  To make e4m3 precision safe (test gate:
rel err < 2e-2):

  - Operands are recentered at zero-point 128 on the host:
    A = x - 128, B = y - 128 in [-128, 127], so the worst e4m3 ulp is
    8 (binade [64,128]) instead of 16 for the raw ranges (x-65 reaches
    190).  Max rounding error per element is 4.
  - The zero-point shift is corrected EXACTLY by rank-1 terms:
      (x-65)@(y-160) = A@B + 63*colsum(B)[j] - 32*rowsum(A)[i] - 2016*K
    Row/col sums are integer-exact on the host and shipped as small
    fp32 vectors; measured sim relmax 7.5e-3 (2.7x margin).

Device kernel:
  - Host stages A (K-major) and B as raw e4m3 bytes; no on-chip
    dequant/cast at all -- DMA feeds the PE directly through SBUF.
  - K is interleaved across SBUF partitions (k = p*kt + j) so each
    load-chunk DMA is 128 large contiguous runs (one per partition);
    a K-permutation applied to both operands leaves the matmul sum
    unchanged.  DoubleRow consumes j-pairs, which are whole k-tile
    pairs of the permuted contraction.
  - x chunks ride the SP HWDGE ring, y chunks alternate ACT/DVE rings;
    correction tiles follow on the SP ring (needed only at the end).
  - PE prewarm releases the HAM clock gate while first chunks land.
  - 128 DoubleRow matmuls (16 k-pairs x 4 m x 2 n) accumulate fp32 in
    all 8 PSUM banks k-outer; the last KP_TAIL pairs run m-major so
    banks retire early and their copy+store overlaps remaining matmuls.
  - The final PSUM->SBUF pass is ONE DVE scalar_tensor_tensor per bank:
      out = (psum * SCALE) + corr[:, m, n-slice]
    where corr[p, m, j] = rvec[m-block][p] + cvec[j] was built on-device
    (early, off the critical path) from the host vectors.
"""

import numpy as np
import ml_dtypes

import concourse.bass as bass  # noqa: F401  (kept for callers/debugging)
import concourse.mybir as mybir
import concourse.tile as tile
from concourse import bacc
from concourse.bass_utils import run_bass_kernel_spmd

X_ZP, Y_ZP = 65.0, 160.0
SCALE = 0.199 * 0.0215
CZP = 128  # recentered zero point: A = x - 128, B = y - 128
XD = CZP - X_ZP  # +63:  (x - 65) = A + 63
YD = CZP - Y_ZP  # -32:  (y - 160) = B - 32

M, K, N = 2048, 4096, 2048
GM, GN = 4, 2  # core grid: 4 M-blocks x 2 N-blocks
MC, NC = M // GM, N // GN  # 512 x 1024 per-core output block
P = 128  # partitions / k-tile size
NB = 512  # psum bank free size (one fp32 bank; matmul cannot cross banks)
DMA_CHUNKS = (2, 2, 4, 8, 8, 8)  # x k-tiles per load DMA (pair-aligned)
# y k-tiles per load DMA: leading chunks on the ACT HWDGE ring; the
# trailing chunk rides the SP ring after the x stream (only 2 HWDGE
# rings exist: SP + ACT), balancing ~3MB per ring.
Y_CHUNKS_ACT = (2, 2, 4, 8, 8)
Y_CHUNKS_SP = (8,)
KP_TAIL = 4  # trailing k-PAIRS run m-major so PSUM banks retire early
N_WARM = 30


def _emit(tc, aT, bs, cb, rv, out, dma_chunks=DMA_CHUNKS,
          y_chunks_act=Y_CHUNKS_ACT, y_chunks_sp=Y_CHUNKS_SP,
          kp_tail=KP_TAIL, n_warm=N_WARM):
    """Emit the per-core device program.

    aT: [k, mc] fp8e4 DRAM (A slice, K-major), bs: [k, nnc] fp8e4 DRAM,
    cb: [P, nnc] fp32 DRAM (col-correction, partition-broadcast),
    rv: [P, mt] fp32 DRAM (row-correction per m-block),
    out: [mc, nnc] fp32 DRAM.
    """
    nc = tc.nc
    k, mc = aT.shape
    nnc = bs.shape[1]
    kt = k // P   # 32 k-tiles
    kp = kt // 2  # 16 DoubleRow k-pairs
    mt = mc // P  # 4
    nt = nnc // NB  # 2
    assert sum(dma_chunks) == kt and all(c % 2 == 0 for c in dma_chunks)
    assert sum(y_chunks_act) + sum(y_chunks_sp) == kt
    assert all(c % 2 == 0 for c in y_chunks_act + y_chunks_sp)

    fp32 = mybir.dt.float32
    f8 = mybir.dt.float8e4
    DR = mybir.MatmulPerfMode.DoubleRow
    MULT, ADD = mybir.AluOpType.mult, mybir.AluOpType.add

    with (
        tc.tile_pool(name="sb", bufs=1) as sbp,
        tc.tile_pool(name="osb", bufs=mt, space="SBUF") as osbp,
        tc.tile_pool(name="ps", bufs=mt * nt, space="PSUM") as psp,
    ):
        # Everything persistent; each DMA writes a disjoint slice.
        au = sbp.tile([P, kt, mc], f8, name="au")
        bu = sbp.tile([P, kt, nnc], f8, name="bu")
        cbt = sbp.tile([P, nnc], fp32, name="cbt")
        rvt = sbp.tile([P, mt], fp32, name="rvt")
        corr = sbp.tile([P, mt, nnc], fp32, name="corr")
        wt = sbp.tile([P, 2, P], f8, name="wt")
        psum = [
            [psp.tile([P, NB], fp32, tag="ps", name=f"ps_{m}_{n}") for n in range(nt)]
            for m in range(mt)
        ]

        # HAM prewarm: throwaway matmuls release the PE clock gate while
        # the first chunks load.
        nc.gpsimd.memset(wt[:], 0.0)
        for _ in range(n_warm):
            nc.tensor.matmul(psum[0][0][:, :P], wt[:], wt[:], start=True,
                             stop=True, perf_mode=DR)

        # K interleaved across partitions (k = p*kt + j): each chunk DMA
        # is 128 contiguous DRAM runs, one per partition.
        aTr = aT.rearrange("(p j) m -> p j m", j=kt)
        bsr = bs.rearrange("(p j) n -> p j n", j=kt)
        k0 = 0
        for nk in dma_chunks:
            nc.sync.dma_start(au[:, k0 : k0 + nk, :], aTr[:, k0 : k0 + nk, :])
            k0 += nk
        k0 = 0
        for nk in y_chunks_act:
            nc.scalar.dma_start(bu[:, k0 : k0 + nk, :], bsr[:, k0 : k0 + nk, :])
            k0 += nk
        # y tail + correction vectors ride the SP ring after the x
        # chunks; they are only needed near the end of the PE stream.
        for nk in y_chunks_sp:
            nc.sync.dma_start(bu[:, k0 : k0 + nk, :], bsr[:, k0 : k0 + nk, :])
            k0 += nk
        nc.sync.dma_start(cbt[:], cb[:])
        nc.sync.dma_start(rvt[:], rv[:])
        # corr[p, m, j] = cvec[j] + rvec_m[p]  (DVE, early, off-path)
        for m in range(mt):
            nc.vector.tensor_scalar_add(corr[:, m, :], cbt[:], rvt[:, m : m + 1])

        def mm(jp, m, n):
            nc.tensor.matmul(
                psum[m][n][:],
                au[:, 2 * jp : 2 * jp + 2, m * P : (m + 1) * P],
                bu[:, 2 * jp : 2 * jp + 2, n * NB : (n + 1) * NB],
                start=(jp == 0),
                stop=(jp == kp - 1),
                perf_mode=DR,
            )

        # k-outer: touch every psum bank each k-pair so the PE stream
        # stays dense while loads race ahead.
        for jp in range(kp - kp_tail):
            for m in range(mt):
                for n in range(nt):
                    mm(jp, m, n)
        # m-outer tail: bank group m finishes its K accumulation early so
        # its copy+store overlaps the remaining matmuls.
        for m in range(mt):
            for jp in range(kp - kp_tail, kp):
                for n in range(nt):
                    mm(jp, m, n)

        # Single DVE pass per bank: out_sb = psum*SCALE + corr.
        for m in range(mt):
            osb = osbp.tile([P, nnc], fp32, tag="osb", name=f"osb_{m}")
            for n in range(nt):
                nc.vector.scalar_tensor_tensor(
                    osb[:, n * NB : (n + 1) * NB],
                    psum[m][n][:],
                    SCALE,
                    corr[:, m, n * NB : (n + 1) * NB],
                    MULT,
                    ADD,
                )
            if m < mt - 1:
                nc.sync.dma_start(out[m * P : (m + 1) * P, :], osb[:])
            else:
                # split the last row-group's store so the kernel-ending
                # chain (last matmul -> copy -> store) is half as long
                for n in range(nt):
                    nc.sync.dma_start(
                        out[m * P : (m + 1) * P, n * NB : (n + 1) * NB],
                        osb[:, n * NB : (n + 1) * NB],
                    )


def _build_nc(k=K, mc=MC, nnc=NC, **emit_kw):
    nc = bacc.Bacc("TRN2", target_bir_lowering=False, debug=False)
    aT = nc.declare_dram_parameter("aT", [k, mc], mybir.dt.float8e4, isOutput=False)
    bs = nc.declare_dram_parameter("bs", [k, nnc], mybir.dt.float8e4, isOutput=False)
    cb = nc.declare_dram_parameter("cb", [P, nnc], mybir.dt.float32, isOutput=False)
    rv = nc.declare_dram_parameter("rv", [P, mc // P], mybir.dt.float32, isOutput=False)
    out = nc.declare_dram_parameter("out", [mc, nnc], mybir.dt.float32, isOutput=True)
    with tile.TileContext(nc) as tc:
        _emit(tc, aT[:], bs[:], cb[:], rv[:], out[:], **emit_kw)
    nc.compile()
    return nc


_CACHE = {}


def _get_nc():
    if "nc" not in _CACHE:
        _CACHE["nc"] = _build_nc()
    return _CACHE["nc"]


def _stage(x, y):
    """Host staging: recentered e4m3 operands + exact rank-1 corrections."""
    f8 = ml_dtypes.float8_e4m3
    a8T = np.ascontiguousarray(
        (x.astype(np.float32) - CZP).astype(f8).T
    )  # [K, M] e4m3 of A = x-128, K-major
    b8 = (y.astype(np.float32) - CZP).astype(f8)  # [K, N]
    # exact integer row/col sums of the recentered operands
    sA = x.sum(axis=1, dtype=np.int64) - CZP * K  # [M]
    sB = y.sum(axis=0, dtype=np.int64) - CZP * K  # [N]
    # (x-65)@(y-160) = A@B + 63*sB[j] - 32*sA[i] + 63*(-32)*K
    rvec = (SCALE * (YD * sA.astype(np.float64) + XD * YD * K)).astype(np.float32)
    cvec = (SCALE * (XD * sB.astype(np.float64))).astype(np.float32)
    return a8T, b8, rvec, cvec


def kernel(x, y):
    x = np.asarray(x)
    y = np.asarray(y)
    assert x.shape == (M, K) and y.shape == (K, N)
    a8T, b8, rvec, cvec = _stage(x, y)

    in_maps = []
    for i in range(GM * GN):
        mi, ni = divmod(i, GN)
        in_maps.append(
            {
                "aT": np.ascontiguousarray(a8T[:, mi * MC : (mi + 1) * MC]),
                "bs": np.ascontiguousarray(b8[:, ni * NC : (ni + 1) * NC]),
                "cb": np.ascontiguousarray(
                    np.broadcast_to(cvec[ni * NC : (ni + 1) * NC], (P, NC))
                ),
                "rv": np.ascontiguousarray(
                    rvec[mi * MC : (mi + 1) * MC].reshape(MC // P, P).T
                ),
            }
        )

    res = run_bass_kernel_spmd(_get_nc(), in_maps, list(range(GM * GN)))
    _CACHE["last_results"] = res

    out = np.empty((M, N), np.float32)
    for i in range(GM * GN):
        mi, ni = divmod(i, GN)
        out[mi * MC : (mi + 1) * MC, ni * NC : (ni + 1) * NC] = res.results[i]["out"]
    return out
